# revision 18
# baseline (speedup 1.0000x reference)
# Trainium2 Bass kernel for CoAttentionModule (axial co-attention, 8 heads).
#
# Sharding: data-parallel over (direction, batch) = 2 x 4 = 8 NeuronCores.
# Core c computes weighted = _coattention(qf, rf)[b].T for its (d, b) pair;
# the host concatenates [features, weighted] per direction.
#
# On-chip layout: the hw axis is w-major everywhere (col = w*32 + i, i = h
# index); the host pre-permutes features and un-permutes the output. This
# makes every matmul stationary operand a contiguous SBUF slice.
#
# Precision: the Q/K projections (and the first O_FP8_CHUNKS chunks of the
# O contraction) run as fp8e4 DoubleRow matmuls: two 128-chunks of the
# contraction ride one instruction at 2x MAC rate. DoubleRowSwInterleave
# mode is essential: plain DoubleRow exposes its 256-row weight load
# (~256 cycles per matmul, measured on hw), while SwInterleave's
# host-interleaved stationary layout (see swi() below) loads like a normal
# matmul and stays hidden under the moving stream. All weights are scaled
# by 64 on host so e4m3 stays clear of its subnormal floor; the scale is
# unwound in the softmax scale (Q and K both carry 64x -> logits carry
# 4096x) and in the output epilogue (psum carries 64x -> DVE mult 1/64 +
# bias add). V carries 64x through sv/av; attT8 copies divide it back out
# so fp8 attT sits in true scale. Error budget (fp64-sim == hw to ~0.1%):
# bf16 base 1.9e-3, +QK fp8 -> 1.68e-2, +O half fp8 -> 1.90e-2 (gate 2e-2).
#
# Per-core pipeline (fp32 PSUM accumulation everywhere):
#   q_all = 64*Wq.T @ xq (+64*bq)  [c_out, hw]   (all 8 heads up front)
#   QAUG[t', (n,w) col i] = sum_c 64*rel[(t'-i)%63, c] q[c, w*32+i]
#       the stationary rolled-rel block is head-independent, so one weight
#       load serves all 8 heads (256 moving rows) per (i, ch).
#   kT = 64*Wk.T @ xr + 64*RWF     RWF[c,(w,k)] = rel[(k-w)%63, c]
#   v  = xr.T @ 64*Wv              [(w,k), c]
#   scores tile (head n, w-group of 4) [128=(w,i), 128=(w,k)]:
#       q.k' + QAUG.KAUG(one-hot) + WIND.KMASK(-1e30 off-diag mask)
#   softmax: exp(scale=1/(16*4096)) with accum_out row sums -> recip -> mul
#   probsT via DVE 32x32 stream transpose (block-diagonal => exact)
#   avT[c,(w,i)] = v.T @ probsT ; outT = (64*Wo).T @ attT * (1/64) + bo'
#   (bv folded on host: bo' = bv @ Wo + bo; bk dropped: softmax-invariant)
import numpy as np
import ml_dtypes

B, C, H, W = 4, 2048, 32, 32
HW = H * W
NH, HD = 8, 256
T = 2 * max(H, W) - 1  # 63
NC = C // 128  # 16 chunks

# ---- precision config ----
QK_FP8 = True      # Q/K projections in fp8 DoubleRow
V_FP8_CHUNKS = 0   # 0..16 (even): contraction chunks of V-proj in fp8 DR
O_FP8_CHUNKS = 8   # 0..16 (even): contraction chunks of O-proj in fp8 DR

SQ = 64.0  # weight upscale, uniform for all projections

F8NP = ml_dtypes.float8_e4m3  # == mybir.dt.np(float8e4)

_CACHE = {}


def _hostprep(Wq, bq, Wk, bk, Wv, bv, Wo, bo, rel_emb):
    bf = ml_dtypes.bfloat16
    f32 = np.float32
    Wq, Wk, Wv, Wo = (np.asarray(a, f32) for a in (Wq, Wk, Wv, Wo))
    rel = np.asarray(rel_emb, f32)  # [63, 256]
    ii = np.arange(32)
    vc, oc = V_FP8_CHUNKS, O_FP8_CHUNKS

    # lhsT blobs [co, p, ci*128+m]: one contiguous [128, *] DMA per co chunk
    def lchunks(Wm, ci_sel=None):
        r = Wm.reshape(NC, 128, NC, 128).transpose(2, 1, 0, 3)  # co p ci m
        if ci_sel is not None:
            r = r[:, :, ci_sel]
        return np.ascontiguousarray(r.reshape(NC, 128, -1))

    def swi(blob):
        # DoubleRowSwInterleave stationary layout: per 256-col pair strip,
        # [A127, B127, A126, B126, ..., A0, B0] (A/B = the two k-tiles,
        # columns reversed). Verified against CoreSim's InstMatmult model.
        co, p, X = blob.shape
        r = blob.reshape(co, p, X // 256, 2, 128)[..., ::-1]
        r = np.swapaxes(r, 3, 4)
        return np.ascontiguousarray(r.reshape(co, p, X))

    out = {}
    if QK_FP8:
        out["wq_l"] = swi(lchunks(Wq * SQ).astype(F8NP))
        out["wk_l"] = swi(lchunks(Wk * SQ).astype(F8NP))
    else:
        out["wq_l"] = lchunks(Wq * SQ).astype(bf)
        out["wk_l"] = lchunks(Wk * SQ).astype(bf)

    # V weights per head [n, p, ci*256+m], split by contraction chunk
    wv5 = (Wv * SQ).reshape(NC, 128, NH, HD).transpose(2, 1, 0, 3)  # n p ci m
    if vc > 0:
        out["wv8_r"] = np.ascontiguousarray(
            wv5[:, :, :vc].reshape(NH, 128, vc * HD)).astype(F8NP)
    if vc < NC:
        out["wvb_r"] = np.ascontiguousarray(
            wv5[:, :, vc:].reshape(NH, 128, -1)).astype(bf)

    # O weights split by contraction chunk: fp8 rows see attT8 (true scale,
    # weights 64x); bf16 rows see attTb (64x scale, weights 1x). Both paths
    # leave psum at 64 * out_true.
    if oc > 0:
        out["wo8_l"] = swi(lchunks(Wo * SQ, ci_sel=slice(0, oc)).astype(F8NP))
    if oc < NC:
        out["wob_l"] = lchunks(Wo, ci_sel=slice(oc, NC)).astype(bf)

    out["bq_c"] = np.ascontiguousarray(
        (np.asarray(bq, f32) * SQ).reshape(NC, 128).T)  # [128,16]
    bo2 = np.asarray(bv, f32) @ Wo + np.asarray(bo, f32)
    out["bo2_c"] = np.ascontiguousarray(bo2.reshape(NC, 128).T)  # [128,16]

    w_idx, k_idx = np.meshgrid(np.arange(32), np.arange(32), indexing="ij")
    # rel_w fold table, w-major [2, 128, 1024]:
    # rwf[ch, p, w*32+k] = SQ * rel[(k-w)%63, ch*128+p]
    rwf = rel[(k_idx - w_idx) % T].reshape(HW, HD) * SQ
    out["rwf"] = np.ascontiguousarray(rwf.T.reshape(2, 128, HW)).astype(bf)
    # rolled rel_emb.T for QAUG: blocks of width 64 (t' padded 63->64):
    # relroll[p, (i,ch)*64 + t'] = SQ * rel[(t'-i)%63, ch*128+p]
    relroll = np.zeros((128, 32 * 2 * 64), f32)
    for i in range(32):
        for ch in range(2):
            blk = rel[(np.arange(T) - i) % T, ch * 128:(ch + 1) * 128] * SQ
            relroll[:, (i * 2 + ch) * 64:(i * 2 + ch) * 64 + T] = blk.T
    out["relroll"] = relroll.astype(bf)
    # key-side aug channels [96, 1024] w-major: rows 0:63 one-hot rel gather
    # (kaug[t, w*32+k] = t==k), row 63 zero, rows 64:96 block-diag mask
    # (kmask[w', w*32+k] = 0 if w==w' else -1e30). Query side: rows 0:63
    # QAUG, row 63 zero, rows 64:96 w-indicator.
    kaug = np.zeros((96, HW), f32)
    kaug[k_idx.reshape(-1), np.arange(HW)] = 1.0
    kaug[64:96] = -1e30
    wind = np.zeros((32, HW), f32)
    for w in range(32):
        wind[w, w * 32 + ii] = 1.0  # query col w*32+i
        kaug[64 + w, w * 32 + ii] = 0.0  # key col w*32+k
    out["kaug"] = kaug.astype(bf)
    out["wind"] = wind.astype(bf)
    return out


def _build(timing_twin=False, loop=1):
    import concourse.bacc as bacc
    import concourse.mybir as mybir
    import concourse.tile as tile

    F32, BF16, F8 = mybir.dt.float32, mybir.dt.bfloat16, mybir.dt.float8e4
    DR = mybir.MatmulPerfMode.DoubleRowSwInterleave
    nc = bacc.Bacc(None, target_bir_lowering=False)

    vc, oc = V_FP8_CHUNKS, O_FP8_CHUNKS
    qk_dt = F8 if QK_FP8 else BF16
    need_xr8 = QK_FP8 or vc > 0
    need_xrb = (not QK_FP8) or vc < NC

    if timing_twin:
        def declare(name, shape, dt, isOutput=False):
            return nc.dram_tensor(name, shape, dt)
        tiny_in = nc.declare_dram_parameter("tiny_in", [1, 4], F32, isOutput=False)
        tiny_out = nc.declare_dram_parameter("tiny_out", [1, 4], F32, isOutput=True)
    else:
        declare = nc.declare_dram_parameter

    xq = declare("xq", [C, HW], qk_dt, isOutput=False)
    if need_xr8:
        xr8 = declare("xr8", [C, HW], F8, isOutput=False)
    if need_xrb:
        xrb = declare("xrb", [C, HW], BF16, isOutput=False)
    wq_l = declare("wq_l", [NC, 128, C], qk_dt, isOutput=False)
    wk_l = declare("wk_l", [NC, 128, C], qk_dt, isOutput=False)
    if vc > 0:
        wv8_r = declare("wv8_r", [NH, 128, vc * HD], F8, isOutput=False)
    if vc < NC:
        wvb_r = declare("wvb_r", [NH, 128, (NC - vc) * HD], BF16, isOutput=False)
    if oc > 0:
        wo8_l = declare("wo8_l", [NC, 128, oc * 128], F8, isOutput=False)
    if oc < NC:
        wob_l = declare("wob_l", [NC, 128, (NC - oc) * 128], BF16, isOutput=False)
    bq_c = declare("bq_c", [128, NC], F32, isOutput=False)
    bo2_c = declare("bo2_c", [128, NC], F32, isOutput=False)
    rwf = declare("rwf", [2, 128, HW], BF16, isOutput=False)
    relroll = declare("relroll", [128, 32 * 2 * 64], BF16, isOutput=False)
    kaug = declare("kaug", [96, HW], BF16, isOutput=False)
    wind = declare("wind", [32, HW], BF16, isOutput=False)
    out = declare("out", [C, HW], F32, isOutput=True)

    EXP = mybir.ActivationFunctionType.Exp
    COPY = mybir.ActivationFunctionType.Copy
    SM_SCALE = 1.0 / (16.0 * SQ * SQ)

    with tile.TileContext(nc) as tc:
        with (
            tc.tile_pool(name="feat", bufs=1) as feat_pool,
            tc.tile_pool(name="att", bufs=1) as att_pool,
            tc.tile_pool(name="const", bufs=1) as const_pool,
            tc.tile_pool(name="head", bufs=2) as head_pool,
            tc.tile_pool(name="wvp", bufs=2) as wvp_pool,
            tc.tile_pool(name="wstr", bufs=4) as wstr_pool,
            tc.tile_pool(name="wstro", bufs=2) as wstro_pool,
            tc.tile_pool(name="probs", bufs=3) as probs_pool,
            tc.tile_pool(name="outs", bufs=2) as outs_pool,
            tc.tile_pool(name="psum", bufs=3, space="PSUM") as psum_pool,
            tc.tile_pool(name="psumb", bufs=3, space="PSUM") as psumb_pool,
            tc.tile_pool(name="psumq", bufs=2, space="PSUM") as psumq_pool,
        ):
            # ---- load features + constants (resident) ----
            xqt = feat_pool.tile([128, NC * HW], qk_dt, tag="xq")
            for cc in range(NC):
                nc.sync.dma_start(xqt[:, cc * HW:(cc + 1) * HW],
                                  xq[cc * 128:(cc + 1) * 128, :])
            if need_xr8:
                xrt8 = feat_pool.tile([128, NC * HW], F8, tag="xr8")
                for cc in range(NC):
                    nc.sync.dma_start(xrt8[:, cc * HW:(cc + 1) * HW],
                                      xr8[cc * 128:(cc + 1) * 128, :])
            if need_xrb:
                xrtb = feat_pool.tile([128, NC * HW], BF16, tag="xrb")
                for cc in range(NC):
                    nc.sync.dma_start(xrtb[:, cc * HW:(cc + 1) * HW],
                                      xrb[cc * 128:(cc + 1) * 128, :])
            q_all = att_pool.tile([128, NC * HW], BF16, tag="qall")
            if oc > 0:
                attT8 = att_pool.tile([128, oc * HW], F8, tag="attT8")
            if oc < NC:
                attTb = att_pool.tile([128, (NC - oc) * HW], BF16, tag="attTb")
            sqa_all = att_pool.tile([96, NH * HW], BF16, tag="sqa")

            c_kaug = const_pool.tile([96, HW], BF16)
            nc.sync.dma_start(c_kaug[:], kaug[:])
            c_wind = const_pool.tile([32, HW], BF16)
            nc.sync.dma_start(c_wind[:], wind[:])
            c_rwf = const_pool.tile([128, 2 * HW], BF16)
            nc.sync.dma_start(c_rwf[:, 0:HW], rwf[0])
            nc.sync.dma_start(c_rwf[:, HW:2 * HW], rwf[1])
            c_roll = const_pool.tile([128, 32 * 2 * 64], BF16)
            nc.sync.dma_start(c_roll[:], relroll[:])
            c_bq = const_pool.tile([128, NC], F32)
            nc.sync.dma_start(c_bq[:], bq_c[:])
            c_bo = const_pool.tile([128, NC], F32)
            nc.sync.dma_start(c_bo[:], bo2_c[:])

            # aug rows 63:96 are rep-invariant: zero pad row + w-indicator
            # (two 32-partition memsets: patterns starting off partition 0
            # may cover at most 32 partitions; rows 32:62 are overwritten by
            # the per-rep QAUG copies anyway)
            nc.vector.memset(sqa_all[32:64, :], 0.0)
            nc.vector.memset(sqa_all[64:96, :], 0.0)
            for n in range(NH):
                nc.vector.tensor_copy(sqa_all[64:96, n * HW:(n + 1) * HW],
                                      c_wind[:])

            def qk_mms(ps, wt, xt3, h2):
                # ps [128,512] += wt.T @ xt[:, :, h2 window]
                if not QK_FP8:
                    for ci in range(NC):
                        nc.tensor.matmul(
                            ps[:], wt[:, ci * 128:(ci + 1) * 128],
                            xt3[:, ci, h2 * 512:(h2 + 1) * 512],
                            start=(ci == 0), stop=(ci == NC - 1))
                else:
                    wt3 = wt[:].rearrange("p (c m) -> p c m", m=128)
                    for j in range(NC // 2):
                        nc.tensor.matmul(
                            ps[:], wt3[:, 2 * j:2 * j + 2, :],
                            xt3[:, 2 * j:2 * j + 2, h2 * 512:(h2 + 1) * 512],
                            start=(j == 0), stop=(j == NC // 2 - 1),
                            perf_mode=DR)

            for rep in range(loop):
                xq3 = xqt[:].rearrange("p (c t) -> p c t", t=HW)
                if need_xr8:
                    xr83 = xrt8[:].rearrange("p (c t) -> p c t", t=HW)
                if need_xrb:
                    xrb3 = xrtb[:].rearrange("p (c t) -> p c t", t=HW)

                # ---- Q projection, all heads ----
                for co in range(NC):
                    wt = wstr_pool.tile([128, C], qk_dt, tag="wl8")
                    nc.sync.dma_start(wt[:], wq_l[co])
                    for h2 in range(2):
                        ps = psum_pool.tile([128, 512], F32, tag="pp")
                        qk_mms(ps, wt, xq3, h2)
                        nc.vector.tensor_scalar_add(
                            q_all[:, co * HW + h2 * 512: co * HW + (h2 + 1) * 512],
                            ps[:], c_bq[:, co:co + 1])

                # ---- QAUG: all heads at once; stationary rel block is
                # head-independent so one weight load serves 256 rows ----
                qa3 = q_all[:].rearrange("p (c t) -> p c t", t=HW)
                sq3 = sqa_all[:].rearrange("p (n t) -> p n t", t=HW)
                for i in range(32):
                    pqa = psumq_pool.tile([64, 256], F32, tag="qa")
                    for ch in range(2):
                        nc.tensor.matmul(
                            pqa[:],
                            c_roll[:, (i * 2 + ch) * 64:(i * 2 + ch + 1) * 64],
                            qa3[:, ch::2, i::32],
                            start=(ch == 0), stop=(ch == 1))
                    nc.vector.tensor_copy(
                        sq3[0:T, :, i::32],
                        pqa[0:T, :].rearrange("p (n w) -> p n w", w=32))

                # ---- per head: K proj, V proj, attention ----
                for n in range(NH):
                    sk = head_pool.tile([128, 2 * HW], BF16, tag="sk")
                    sv = head_pool.tile([128, NH * HD], BF16, tag="sv")
                    if vc > 0:
                        swv8 = wvp_pool.tile([128, vc * HD], F8, tag="swv8")
                        nc.sync.dma_start(swv8[:], wv8_r[n])
                    nbv = NC - vc
                    mid = (nbv + 1) // 2
                    if vc < NC:
                        swvb_a = wvp_pool.tile([128, mid * HD], BF16,
                                               tag="swvba")
                        nc.sync.dma_start(swvb_a[:], wvb_r[n, :, 0:mid * HD])
                        if nbv > mid:
                            swvb_b = wvp_pool.tile([128, (nbv - mid) * HD],
                                                   BF16, tag="swvbb")
                            nc.sync.dma_start(swvb_b[:],
                                              wvb_r[n, :, mid * HD:nbv * HD])

                    for co2 in range(2):
                        co = n * 2 + co2
                        wt = wstr_pool.tile([128, C], qk_dt, tag="wl8")
                        nc.sync.dma_start(wt[:], wk_l[co])
                        for h2 in range(2):
                            ps = psum_pool.tile([128, 512], F32, tag="pp")
                            qk_mms(ps, wt, xr83 if QK_FP8 else xrb3, h2)
                            dpos = sk[:, co2 * HW + h2 * 512:
                                      co2 * HW + (h2 + 1) * 512]
                            nc.vector.tensor_add(
                                dpos, ps[:],
                                c_rwf[:, co2 * HW + h2 * 512:
                                      co2 * HW + (h2 + 1) * 512])

                    # V projection, w-major rows; psv = 64 * v
                    nmm = vc // 2 + (NC - vc)
                    for wg in range(8):
                        psv = psum_pool.tile([128, HD], F32, tag="pp")
                        done = 0
                        if vc > 0:
                            sv83 = swv8[:].rearrange("p (c m) -> p c m", m=HD)
                            for j in range(vc // 2):
                                nc.tensor.matmul(
                                    psv[:],
                                    xr83[:, 2 * j:2 * j + 2,
                                         wg * 128:(wg + 1) * 128],
                                    sv83[:, 2 * j:2 * j + 2, :],
                                    start=(done == 0), stop=(done == nmm - 1),
                                    perf_mode=DR)
                                done += 1
                        if vc < NC:
                            svb3a = swvb_a[:].rearrange("p (c m) -> p c m", m=HD)
                            if nbv > mid:
                                svb3b = swvb_b[:].rearrange(
                                    "p (c m) -> p c m", m=HD)
                            for ci in range(nbv):
                                sl = (svb3a[:, ci] if ci < mid
                                      else svb3b[:, ci - mid])
                                nc.tensor.matmul(
                                    psv[:],
                                    xrb3[:, vc + ci, wg * 128:(wg + 1) * 128],
                                    sl,
                                    start=(done == 0), stop=(done == nmm - 1))
                                done += 1
                        nc.vector.tensor_copy(sv[:, wg * HD:(wg + 1) * HD],
                                              psv[:])

                    # attention per w-group, software-pipelined: av matmuls
                    # for group wg issue after the score matmuls of wg+1, so
                    # the in-order PE never waits on the softmax chain.
                    def issue_av(pT, pwg):
                        for ch in range(2):
                            av = psumb_pool.tile([128, 128], F32, tag="sa")
                            nc.tensor.matmul(
                                av[:],
                                sv[:, pwg * HD + ch * 128:
                                   pwg * HD + (ch + 1) * 128],
                                pT[:], start=True, stop=True)
                            cc = n * 2 + ch
                            if cc < oc:
                                # attT8 holds true-scale att (psum is 64x)
                                nc.scalar.activation(
                                    attT8[:, cc * HW + pwg * 128:
                                          cc * HW + (pwg + 1) * 128],
                                    av[:], COPY, scale=1.0 / SQ)
                            else:
                                nc.vector.tensor_copy(
                                    attTb[:, (cc - oc) * HW + pwg * 128:
                                          (cc - oc) * HW + (pwg + 1) * 128],
                                    av[:])

                    pend = None
                    for wg in range(8):
                        sc = psumb_pool.tile([128, 128], F32, tag="sa")
                        nc.tensor.matmul(
                            sc[:],
                            q_all[:, (2 * n) * HW + wg * 128:
                                  (2 * n) * HW + (wg + 1) * 128],
                            sk[:, wg * 128:(wg + 1) * 128],
                            start=True, stop=False)
                        nc.tensor.matmul(
                            sc[:],
                            q_all[:, (2 * n + 1) * HW + wg * 128:
                                  (2 * n + 1) * HW + (wg + 1) * 128],
                            sk[:, HW + wg * 128: HW + (wg + 1) * 128],
                            start=False, stop=False)
                        nc.tensor.matmul(
                            sc[:],
                            sqa_all[:, n * HW + wg * 128:
                                    n * HW + (wg + 1) * 128],
                            c_kaug[:, wg * 128:(wg + 1) * 128],
                            start=False, stop=True)
                        if pend is not None:
                            issue_av(*pend)
                        probs = probs_pool.tile([128, 128], BF16, tag="pr")
                        sums = probs_pool.tile([128, 1], F32, tag="sm")
                        recip = probs_pool.tile([128, 1], F32, tag="rc")
                        nc.scalar.activation(probs[:], sc[:], EXP,
                                             scale=SM_SCALE, accum_out=sums[:])
                        nc.vector.reciprocal(recip[:], sums[:])
                        nc.vector.tensor_scalar_mul(probs[:], probs[:], recip[:])
                        probsT = probs_pool.tile([128, 128], BF16, tag="prT")
                        nc.vector.transpose(probsT[:], probs[:])
                        pend = (probsT, wg)
                    issue_av(*pend)

                # ---- output projection: psum = 64 * out_true ----
                if oc > 0:
                    at83 = attT8[:].rearrange("p (c t) -> p c t", t=HW)
                if oc < NC:
                    atb3 = attTb[:].rearrange("p (c t) -> p c t", t=HW)
                nmo = oc // 2 + (NC - oc)
                for co in range(NC):
                    if oc > 0:
                        wt8 = wstro_pool.tile([128, oc * 128], F8, tag="wo8")
                        nc.sync.dma_start(wt8[:], wo8_l[co])
                    if oc < NC:
                        wtb = wstro_pool.tile([128, (NC - oc) * 128], BF16,
                                              tag="wob")
                        nc.sync.dma_start(wtb[:], wob_l[co])
                    for h2 in range(2):
                        ps = psum_pool.tile([128, 512], F32, tag="pp")
                        done = 0
                        if oc > 0:
                            wt83 = wt8[:].rearrange("p (c m) -> p c m", m=128)
                            for j in range(oc // 2):
                                nc.tensor.matmul(
                                    ps[:], wt83[:, 2 * j:2 * j + 2, :],
                                    at83[:, 2 * j:2 * j + 2,
                                         h2 * 512:(h2 + 1) * 512],
                                    start=(done == 0), stop=(done == nmo - 1),
                                    perf_mode=DR)
                                done += 1
                        if oc < NC:
                            wtb3 = wtb[:].rearrange("p (c m) -> p c m", m=128)
                            for ci in range(NC - oc):
                                nc.tensor.matmul(
                                    ps[:], wtb3[:, ci],
                                    atb3[:, ci, h2 * 512:(h2 + 1) * 512],
                                    start=(done == 0), stop=(done == nmo - 1))
                                done += 1
                        ot = outs_pool.tile([128, 512], F32, tag="ot")
                        nc.vector.tensor_scalar(
                            ot[:], ps[:], 1.0 / SQ, c_bo[:, co:co + 1],
                            op0=mybir.AluOpType.mult, op1=mybir.AluOpType.add)
                        nc.sync.dma_start(
                            out[co * 128:(co + 1) * 128,
                                h2 * 512:(h2 + 1) * 512], ot[:])

                if timing_twin:
                    tt = outs_pool.tile([1, 4], F32, tag="tt")
                    nc.sync.dma_start(tt[:], tiny_in[:])
                    nc.sync.dma_start(tiny_out[:], tt[:])

            if timing_twin:
                tt = outs_pool.tile([1, 4], F32, tag="tt")
                nc.sync.dma_start(tt[:], tiny_in[:])
                nc.sync.dma_start(tiny_out[:], tt[:])

    nc.finalize()
    return nc


def kernel(left_features, right_features, Wq, bq, Wk, bk, Wv, bv, Wo, bo, rel_emb,
           _trace=False):
    from concourse.bass_utils import run_bass_kernel_spmd

    bf = ml_dtypes.bfloat16
    if "nc" not in _CACHE:
        _CACHE["nc"] = _build()
    nc = _CACHE["nc"]

    consts = _hostprep(Wq, bq, Wk, bk, Wv, bv, Wo, bo, rel_emb)
    lf = np.asarray(left_features, np.float32)
    rf = np.asarray(right_features, np.float32)

    vc = V_FP8_CHUNKS
    need_xr8 = QK_FP8 or vc > 0
    need_xrb = (not QK_FP8) or vc < NC

    def wmajor(x):  # (C, H, W) -> (C, HW) with col = w*32 + i
        return np.ascontiguousarray(x.transpose(0, 2, 1).reshape(C, HW))

    in_maps = []
    for core in range(8):
        d, b = divmod(core, 4)
        qf = lf[b] if d == 0 else rf[b]
        rfb = rf[b] if d == 0 else lf[b]
        m = dict(consts)
        xq_w = wmajor(qf)
        xr_w = wmajor(rfb)
        m["xq"] = xq_w.astype(F8NP if QK_FP8 else bf)
        if need_xr8:
            m["xr8"] = xr_w.astype(F8NP)
        if need_xrb:
            m["xrb"] = xr_w.astype(bf)
        in_maps.append(m)

    res = run_bass_kernel_spmd(nc, in_maps, list(range(8)), trace=_trace)
    _CACHE["last_result"] = res

    def unperm(o):  # [C, HW w-major] -> (C, H, W)
        return np.ascontiguousarray(o.reshape(C, W, H).transpose(0, 2, 1))

    wr = np.stack([unperm(res.results[b]["out"]) for b in range(4)])
    wl = np.stack([unperm(res.results[4 + b]["out"]) for b in range(4)])
    left_att = np.concatenate([lf, wr], axis=1)
    right_att = np.concatenate([rf, wl], axis=1)
    return (left_att, right_att)
